# revision 29
# baseline (speedup 1.0000x reference)
"""ArcFace loss on 8 TRN2 NeuronCores (vocab/tensor-parallel over classes).

Math (per reference):
    cos = normalize(emb) @ normalize(W).T            [B, C]
    phi applied at the label column only (ArcFace margin)
    loss = mean CE(64 * modified cos, labels)

Device-side work is reduced to the two irreducible O(B*C) pieces: the
big cosine matmul and the per-row sum of exp(64*cos - 16).  Everything
else is O(B*D) or O(C*D) staging done on the host:

  host stage:  normalize rows of emb and W, scale by 16, cast to
               fp8e4m3, lay out transposed (contraction dim on
               partitions) for the PE; per-core class shard padded to
               12800 = 25 blocks of 512.
  device:      for each class-block: 2 DoubleRow fp8 matmuls
               (K=256 each) accumulating into PSUM, then one
               Activation Exp over a 4-bank super-block with
               accum_out producing per-row partial sums.  The only
               output is a [128, 8] tile of per-row partial sum-exps
               (with the constant -16 flash bias folded in).
  host final:  sum partials across the 8 cores, apply the exact fp32
               label-column correction (replace exp(64*cos_l) by
               exp(64*phi_l)), subtract the zero-pad contribution,
               take log and the batch mean.

The fp8 (e4m3, TRN max 240) quantization of the two normalized
operands gives ~1.7e-3 cosine noise -> ~1e-4 relative loss error,
far inside the 2e-2 gate, and doubles PE throughput via DoubleRow.
"""

import math
import numpy as np
import ml_dtypes

import concourse.mybir as mybir
from concourse import bacc, tile
from concourse.bass_utils import run_bass_kernel_spmd

N_CORES = 8
B = 1024
D = 512
C = 100000
C_PER = C // N_CORES          # 12500
CP = 12800                    # per-core classes padded to 25 * 512
CB = 512                      # matmul free-dim block (one PSUM bank)
SUPER_CB = 4                  # class blocks per exp super-block (4 banks)
SCALE = 64.0
MARGIN = 0.5
EXP_BIAS = -16.0
SE = 16.0                     # fp8 pre-scale for normalized embeddings
SW = 16.0                     # fp8 pre-scale for normalized weights

M_TILES = B // 128            # 8
K_CHUNKS = D // 128           # 4
K_PAIRS = K_CHUNKS // 2       # 2 DoubleRow K=256 chunks
N_BLOCKS = CP // CB           # 25

# Schraudolph fast-exp constants for the DVE offload path:
#   exp(64*cos - 16) = 2^t,  t = psum*(log2e/4) - 16*log2e  (psum = 256*cos)
#   I = round(A_SCH*psum + B_SCH) as int32, bitcast to fp32 ~= 2^t
# C_SCH calibrated so the sum over a lognormal-ish exp distribution is
# unbiased (ratio 0.99986 in simulation).
LOG2E = math.log2(math.e)
C_SCH = 486411.0
A_SCH = float(np.float32(2.0 ** 23 * LOG2E / 4.0))
B_SCH = float(np.float32(2.0 ** 23 * (127.0 - 16.0 * LOG2E) - C_SCH))

# PSUM is carved into a depth-3 ring: A = banks 0-2, B = banks 3-5,
# D = banks 6-7, cycled per chunk in m-major order.  Ring window before
# a region must be refilled = fill(other two) = 2160ns, which hides the
# ~600ns drain-completion ack the 2-buffer layout serialized.  Each
# 3-bank chunk is drained split: ACT exps [0:ACT_SPLIT], the DVE
# Schraudolph-exps the rest; the 2-bank D chunk is ACT-only.
CHUNK_SEQ = [("A", 3), ("A", 3), ("D", 2)] * 3 + [("A", 1)]
ACT_SPLIT = 1000
CB_LAST = 256                 # last block computes only 256 of its 512 cols
CP_EFF = (N_BLOCKS - 1) * CB + CB_LAST   # 12544 >= 12500 real classes

FP32 = mybir.dt.float32
BF16 = mybir.dt.bfloat16
FP8 = mybir.dt.float8e4
AF = mybir.ActivationFunctionType
ALU = mybir.AluOpType
X = mybir.AxisListType.X
PERF = mybir.MatmulPerfMode.DoubleRow

COS_M = math.cos(MARGIN)
SIN_M = math.sin(MARGIN)
TH = math.cos(math.pi - MARGIN)
MM = math.sin(math.pi - MARGIN) * MARGIN


def _supers(n_blocks: int, super_cb: int):
    """[(first_block, n_cb), ...] covering n_blocks class blocks."""
    out = []
    b = 0
    while b < n_blocks:
        n = min(super_cb, n_blocks - b)
        out.append((b, n))
        b += n
    return out


def build_graph(b=B, cp=CP, super_cb=SUPER_CB):
    m_tiles = b // 128
    n_blocks = cp // CB
    supers = _supers(n_blocks, super_cb)
    n_sup = len(supers)

    nc = bacc.Bacc("TRN2", target_bir_lowering=False, debug=False,
                   num_devices=N_CORES)
    # ehT: normalized*SE embeddings, transposed: [p, k, b] = ehat[b, k*128+p]
    ehT_d = nc.dram_tensor("ehT", [128, K_CHUNKS * b], FP8,
                           kind="ExternalInput")
    # wt: per-core shard, block-major: [(cb p), (k x)] = what[cb*512+x, k*128+p]
    wt_d = nc.dram_tensor("wt", [n_blocks * 128, K_CHUNKS * CB], FP8,
                          kind="ExternalInput")
    out_d = nc.dram_tensor("out", [128, 2 * m_tiles], FP32,
                           kind="ExternalOutput")

    ehT_ap = ehT_d.ap()
    wt_ap = wt_d.ap()
    INT32 = mybir.dt.int32

    with tile.TileContext(nc) as tc:
        with (
            tc.tile_pool(name="persist", bufs=1) as pp,
            tc.tile_pool(name="wpool", bufs=n_sup) as wp,
            tc.tile_pool(name="scpool", bufs=3) as sc_p,
            tc.tile_pool(name="small", bufs=2) as sp,
            tc.tile_pool(name="psA", bufs=2, space="PSUM") as psA,
            tc.tile_pool(name="psD", bufs=1, space="PSUM") as psD,
        ):
            bias_n = pp.tile([128, 1], FP32, tag="bias_n")
            nc.vector.memset(bias_n[:], EXP_BIAS)
            # Prewarm the exp activation-table set so the ~2.7us table
            # load overlaps the input DMA instead of the first real exp.
            warm = pp.tile([128, 1], FP32, tag="warm")
            nc.scalar.activation(warm[:], bias_n[:], AF.Exp)

            ehT = pp.tile([128, K_CHUNKS, b], FP8, tag="ehT")
            nc.scalar.dma_start(
                ehT[:], ehT_ap[:, :].rearrange("p (k x) -> p k x",
                                               k=K_CHUNKS))
            accA = [pp.tile([128, 10], FP32, tag=f"accA{m}",
                            name=f"accA{m}") for m in range(m_tiles)]
            accD = [pp.tile([128, 6], FP32, tag=f"accD{m}",
                            name=f"accD{m}") for m in range(m_tiles)]

            # whole weight shard resident; super 0 split 4 ways across
            # queues so block 0 lands early, rest streamed in order
            wh = {}

            def fetch(si, queues):
                cb0, ncb = supers[si]
                wt_t = wp.tile([128, super_cb, K_CHUNKS, CB], FP8, tag="wt",
                               name=f"wt{si}")
                nq = min(len(queues), ncb)
                per = ncb // nq
                for qi, q in enumerate(queues[:nq]):
                    lo = qi * per
                    hi = ncb if qi == nq - 1 else (qi + 1) * per
                    q.dma_start(
                        wt_t[:, lo:hi, :, :],
                        wt_ap[(cb0 + lo) * 128:(cb0 + hi) * 128, :].rearrange(
                            "(c p) (k x) -> p c k x", p=128, k=K_CHUNKS))
                wh[si] = wt_t

            fetch(0, (nc.sync, nc.sync, nc.gpsimd, nc.gpsimd))
            qsched = {1: nc.sync, 2: nc.scalar, 3: nc.gpsimd,
                      4: nc.sync, 5: nc.scalar, 6: nc.gpsimd}
            for si in range(1, n_sup):
                fetch(si, (qsched[si],))

            colA = [0] * m_tiles
            colD = [0] * m_tiles
            for m in range(m_tiles):
                blk = 0
                for kind, nb in CHUNK_SEQ:
                    if kind == "A":
                        pg = psA.tile([128, 3 * CB], FP32, tag="pgA",
                                      name=f"pgA{m}_{blk}")
                    else:
                        pg = psD.tile([128, 2 * CB], FP32, tag="pgD",
                                      name=f"pgD{m}_{blk}")
                    for bi in range(nb):
                        cb = blk + bi
                        wt_t = wh[cb // super_cb]
                        w = CB_LAST if cb == n_blocks - 1 else CB
                        for j in range(K_PAIRS):
                            nc.tensor.matmul(
                                pg[:, bi * CB:bi * CB + w],
                                ehT[:, 2 * j:2 * j + 2,
                                    m * 128:(m + 1) * 128],
                                wt_t[:, cb % super_cb, 2 * j:2 * j + 2, :w],
                                start=(j == 0), stop=(j == K_PAIRS - 1),
                                perf_mode=PERF)
                    alen = (nb - 1) * CB + w
                    if kind == "A" and nb == 3:
                        dn = 3 * CB - ACT_SPLIT
                        sc = sc_p.tile([128, dn], INT32, tag="sc",
                                       name=f"sc{m}_{blk}")
                        nc.vector.tensor_scalar(
                            sc[:], pg[:, ACT_SPLIT:], A_SCH, B_SCH,
                            ALU.mult, ALU.add)
                        ci = colD[m]; colD[m] += 1
                        nc.vector.tensor_reduce(
                            accD[m][:, ci:ci + 1], sc[:].bitcast(FP32),
                            X, ALU.add)
                        alen = ACT_SPLIT
                    # exp written back in place over the PSUM chunk
                    ci = colA[m]; colA[m] += 1
                    nc.scalar.activation(
                        pg[:, :alen], pg[:, :alen], AF.Exp,
                        bias=bias_n[:], scale=SCALE / (SE * SW),
                        accum_out=accA[m][:, ci:ci + 1])
                    blk += nb

            sred = sp.tile([128, 2 * m_tiles], FP32, tag="sred")
            for m in range(m_tiles):
                nc.vector.tensor_reduce(sred[:, m:m + 1],
                                        accA[m][:, :colA[m]], X, ALU.add)
                dcol = sred[:, m_tiles + m:m_tiles + m + 1]
                if colD[m] > 0:
                    nc.vector.tensor_reduce(dcol, accD[m][:, :colD[m]],
                                            X, ALU.add)
                else:
                    nc.vector.memset(dcol, 0.0)
            nc.sync.dma_start(out_d.ap()[:, :], sred[:])

    nc.compile()
    return nc


def make_in_maps(embeddings, weight, labels, b=B, cp=CP):
    """Host staging: normalize, fp8-cast, transpose, shard; plus the fp64
    label-correction context used by finalize()."""
    emb = np.asarray(embeddings, np.float64)
    w = np.asarray(weight, np.float64)
    lab = np.asarray(labels).astype(np.int64)
    c, d = w.shape

    ehat = emb / np.maximum(np.linalg.norm(emb, axis=1, keepdims=True), 1e-12)
    what = w / np.maximum(np.linalg.norm(w, axis=1, keepdims=True), 1e-12)

    e8 = (ehat * SE).astype(np.float32).astype(ml_dtypes.float8_e4m3)
    # [B, D] -> [B, K, 128] -> [128, K, B]
    ehT8 = np.ascontiguousarray(
        e8.reshape(b, K_CHUNKS, 128).transpose(2, 1, 0)).reshape(128, -1)

    w8 = (what * SW).astype(np.float32).astype(ml_dtypes.float8_e4m3)
    c_per = c // N_CORES
    in_maps = []
    for i in range(N_CORES):
        shard = np.zeros((cp, d), ml_dtypes.float8_e4m3)
        shard[:c_per] = w8[i * c_per:(i + 1) * c_per]
        # [CP, D] -> [NB, 512, K, 128] -> [NB, 128, K, 512] -> 2D
        wt8 = np.ascontiguousarray(
            shard.reshape(N_BLOCKS, CB, K_CHUNKS, 128).transpose(0, 3, 2, 1)
        ).reshape(N_BLOCKS * 128, K_CHUNKS * CB)
        in_maps.append({"ehT": ehT8, "wt": wt8})

    # fp64 label-column correction (exact cos at the label position)
    cos_l = np.einsum('bd,bd->b', ehat, what[lab])
    sin_l = np.sqrt(np.clip(1.0 - cos_l * cos_l, 0.0, 1.0))
    phi = cos_l * COS_M - sin_l * SIN_M
    phi = np.where(cos_l > TH, phi, cos_l - MM)
    t = SCALE * phi
    delta = np.exp(t + EXP_BIAS) - np.exp(SCALE * cos_l + EXP_BIAS)
    n_pad_total = (CP_EFF - c_per) * N_CORES
    host_ctx = {"t": t, "delta": delta,
                "pad": n_pad_total * math.exp(EXP_BIAS)}
    return in_maps, host_ctx


def finalize(core_outs, host_ctx, b=B):
    """core_outs: list of [128, 2*M_TILES] per-core partial sum-exp tiles
    (ACT-exact columns 0..7, DVE-Schraudolph columns 8..15)."""
    total = np.zeros((128, 2 * M_TILES), np.float64)
    for o in core_outs:
        total += np.asarray(o, np.float64)
    merged = total[:, :M_TILES] + total[:, M_TILES:]
    # row b = m*128 + p  ->  flatten [p, m] with order p-major per column
    sum_dev = merged.transpose(1, 0).reshape(b)
    sum_all = sum_dev + host_ctx["delta"] - host_ctx["pad"]
    loss = np.mean(np.log(sum_all) - EXP_BIAS - host_ctx["t"])
    return np.float32(loss)


_CACHED_NC = None


def kernel(embeddings, weight, labels):
    global _CACHED_NC
    if _CACHED_NC is None:
        _CACHED_NC = build_graph()
    in_maps, host_ctx = make_in_maps(embeddings, weight, labels)
    res = run_bass_kernel_spmd(_CACHED_NC, in_maps,
                               core_ids=list(range(N_CORES)), trace=False)
    return finalize([r["out"] for r in res.results], host_ctx)


if __name__ == "__main__":
    rng = np.random.default_rng(0)
    e = rng.standard_normal((B, D)).astype(np.float32)
    w = (rng.random((C, D)).astype(np.float32) - 0.5) * 0.015
    l = rng.integers(0, C, B).astype(np.int64)
    print(kernel(e, w, l))


# revision 30
# speedup vs baseline: 1.0314x; 1.0314x over previous
"""ArcFace loss on 8 TRN2 NeuronCores (vocab/tensor-parallel over classes).

Math (per reference):
    cos = normalize(emb) @ normalize(W).T            [B, C]
    phi applied at the label column only (ArcFace margin)
    loss = mean CE(64 * modified cos, labels)

Device-side work is reduced to the two irreducible O(B*C) pieces: the
big cosine matmul and the per-row sum of exp(64*cos - 16).  Everything
else is O(B*D) or O(C*D) staging done on the host:

  host stage:  normalize rows of emb and W, scale by 16, cast to
               fp8e4m3, lay out transposed (contraction dim on
               partitions) for the PE; per-core class shard padded to
               12800 = 25 blocks of 512.
  device:      for each class-block: 2 DoubleRow fp8 matmuls
               (K=256 each) accumulating into PSUM, then one
               Activation Exp over a 4-bank super-block with
               accum_out producing per-row partial sums.  The only
               output is a [128, 8] tile of per-row partial sum-exps
               (with the constant -16 flash bias folded in).
  host final:  sum partials across the 8 cores, apply the exact fp32
               label-column correction (replace exp(64*cos_l) by
               exp(64*phi_l)), subtract the zero-pad contribution,
               take log and the batch mean.

The fp8 (e4m3, TRN max 240) quantization of the two normalized
operands gives ~1.7e-3 cosine noise -> ~1e-4 relative loss error,
far inside the 2e-2 gate, and doubles PE throughput via DoubleRow.
"""

import math
import numpy as np
import ml_dtypes

import concourse.mybir as mybir
from concourse import bacc, tile
from concourse.bass_utils import run_bass_kernel_spmd

N_CORES = 8
B = 1024
D = 512
C = 100000
C_PER = C // N_CORES          # 12500
CP = 12800                    # per-core classes padded to 25 * 512
CB = 512                      # matmul free-dim block (one PSUM bank)
SUPER_CB = 4                  # class blocks per exp super-block (4 banks)
SCALE = 64.0
MARGIN = 0.5
EXP_BIAS = -16.0
SE = 16.0                     # fp8 pre-scale for normalized embeddings
SW = 16.0                     # fp8 pre-scale for normalized weights

M_TILES = B // 128            # 8
K_CHUNKS = D // 128           # 4
K_PAIRS = K_CHUNKS // 2       # 2 DoubleRow K=256 chunks
N_BLOCKS = CP // CB           # 25

# Schraudolph fast-exp constants for the DVE offload path:
#   exp(64*cos - 16) = 2^t,  t = psum*(log2e/4) - 16*log2e  (psum = 256*cos)
#   I = round(A_SCH*psum + B_SCH) as int32, bitcast to fp32 ~= 2^t
# C_SCH calibrated so the sum over a lognormal-ish exp distribution is
# unbiased (ratio 0.99986 in simulation).
LOG2E = math.log2(math.e)
C_SCH = 486411.0
A_SCH = float(np.float32(2.0 ** 23 * LOG2E / 4.0))
B_SCH = float(np.float32(2.0 ** 23 * (127.0 - 16.0 * LOG2E) - C_SCH))

# PSUM is carved into a depth-3 ring: A = banks 0-2, B = banks 3-5,
# D = banks 6-7, cycled per chunk in m-major order.  Ring window before
# a region must be refilled = fill(other two) = 2160ns, which hides the
# ~600ns drain-completion ack the 2-buffer layout serialized.  Each
# 3-bank chunk is drained split: ACT exps [0:ACT_SPLIT], the DVE
# Schraudolph-exps the rest; the 2-bank D chunk is ACT-only.
CHUNK_SEQ = [("A", 3), ("A", 3), ("D", 2)] * 3 + [("A", 1)]
ACT_SPLIT = 1000
CB_LAST = 256                 # last block computes only 256 of its 512 cols
CP_EFF = (N_BLOCKS - 1) * CB + CB_LAST   # 12544 >= 12500 real classes

FP32 = mybir.dt.float32
BF16 = mybir.dt.bfloat16
FP8 = mybir.dt.float8e4
AF = mybir.ActivationFunctionType
ALU = mybir.AluOpType
X = mybir.AxisListType.X
PERF = mybir.MatmulPerfMode.DoubleRow

COS_M = math.cos(MARGIN)
SIN_M = math.sin(MARGIN)
TH = math.cos(math.pi - MARGIN)
MM = math.sin(math.pi - MARGIN) * MARGIN


def _supers(n_blocks: int, super_cb: int):
    """[(first_block, n_cb), ...] covering n_blocks class blocks."""
    out = []
    b = 0
    while b < n_blocks:
        n = min(super_cb, n_blocks - b)
        out.append((b, n))
        b += n
    return out


def build_graph(b=B, cp=CP, super_cb=SUPER_CB):
    m_tiles = b // 128
    n_blocks = cp // CB
    supers = _supers(n_blocks, super_cb)
    n_sup = len(supers)

    nc = bacc.Bacc("TRN2", target_bir_lowering=False, debug=False,
                   num_devices=N_CORES)
    # ehT: normalized*SE embeddings, transposed: [p, k, b] = ehat[b, k*128+p]
    ehT_d = nc.dram_tensor("ehT", [128, K_CHUNKS * b], FP8,
                           kind="ExternalInput")
    # wt: per-core shard, block-major: [(cb p), (k x)] = what[cb*512+x, k*128+p]
    wt_d = nc.dram_tensor("wt", [n_blocks * 128, K_CHUNKS * CB], FP8,
                          kind="ExternalInput")
    out_d = nc.dram_tensor("out", [128, 2 * m_tiles], FP32,
                           kind="ExternalOutput")

    ehT_ap = ehT_d.ap()
    wt_ap = wt_d.ap()
    INT32 = mybir.dt.int32

    with tile.TileContext(nc) as tc:
        with (
            tc.tile_pool(name="persist", bufs=1) as pp,
            tc.tile_pool(name="wpool", bufs=n_sup) as wp,
            tc.tile_pool(name="scpool", bufs=3) as sc_p,
            tc.tile_pool(name="small", bufs=2) as sp,
            tc.tile_pool(name="psA", bufs=2, space="PSUM") as psA,
            tc.tile_pool(name="psD", bufs=1, space="PSUM") as psD,
        ):
            bias_n = pp.tile([128, 1], FP32, tag="bias_n")
            nc.vector.memset(bias_n[:], EXP_BIAS)
            # Prewarm the exp activation-table set so the ~2.7us table
            # load overlaps the input DMA instead of the first real exp.
            warm = pp.tile([128, 1], FP32, tag="warm")
            nc.scalar.activation(warm[:], bias_n[:], AF.Exp)

            ehT = pp.tile([128, K_CHUNKS, b], FP8, tag="ehT")
            nc.scalar.dma_start(
                ehT[:], ehT_ap[:, :].rearrange("p (k x) -> p k x",
                                               k=K_CHUNKS))
            accA = [pp.tile([128, 10], FP32, tag=f"accA{m}",
                            name=f"accA{m}") for m in range(m_tiles)]
            accD = [pp.tile([128, 6], FP32, tag=f"accD{m}",
                            name=f"accD{m}") for m in range(m_tiles)]

            # whole weight shard resident; super 0 split 4 ways across
            # queues so block 0 lands early, rest streamed in order
            wh = {}

            def fetch(si, queues):
                cb0, ncb = supers[si]
                wt_t = wp.tile([128, super_cb, K_CHUNKS, CB], FP8, tag="wt",
                               name=f"wt{si}")
                nq = min(len(queues), ncb)
                per = ncb // nq
                for qi, q in enumerate(queues[:nq]):
                    lo = qi * per
                    hi = ncb if qi == nq - 1 else (qi + 1) * per
                    q.dma_start(
                        wt_t[:, lo:hi, :, :],
                        wt_ap[(cb0 + lo) * 128:(cb0 + hi) * 128, :].rearrange(
                            "(c p) (k x) -> p c k x", p=128, k=K_CHUNKS))
                wh[si] = wt_t

            fetch(0, (nc.sync, nc.sync, nc.gpsimd, nc.gpsimd))
            for si in range(1, n_sup):
                fetch(si, (nc.sync,) if si % 2 else (nc.gpsimd,))

            colA = [0] * m_tiles
            colD = [0] * m_tiles
            for m in range(m_tiles):
                blk = 0
                for kind, nb in CHUNK_SEQ:
                    if kind == "A":
                        pg = psA.tile([128, 3 * CB], FP32, tag="pgA",
                                      name=f"pgA{m}_{blk}")
                    else:
                        pg = psD.tile([128, 2 * CB], FP32, tag="pgD",
                                      name=f"pgD{m}_{blk}")
                    for bi in range(nb):
                        cb = blk + bi
                        wt_t = wh[cb // super_cb]
                        w = CB_LAST if cb == n_blocks - 1 else CB
                        for j in range(K_PAIRS):
                            nc.tensor.matmul(
                                pg[:, bi * CB:bi * CB + w],
                                ehT[:, 2 * j:2 * j + 2,
                                    m * 128:(m + 1) * 128],
                                wt_t[:, cb % super_cb, 2 * j:2 * j + 2, :w],
                                start=(j == 0), stop=(j == K_PAIRS - 1),
                                perf_mode=PERF)
                    alen = (nb - 1) * CB + w
                    if kind == "A" and nb == 3:
                        dn = 3 * CB - ACT_SPLIT
                        sc = sc_p.tile([128, dn], INT32, tag="sc",
                                       name=f"sc{m}_{blk}")
                        nc.vector.tensor_scalar(
                            sc[:], pg[:, ACT_SPLIT:], A_SCH, B_SCH,
                            ALU.mult, ALU.add)
                        ci = colD[m]; colD[m] += 1
                        nc.vector.tensor_reduce(
                            accD[m][:, ci:ci + 1], sc[:].bitcast(FP32),
                            X, ALU.add)
                        alen = ACT_SPLIT
                    # exp written back in place over the PSUM chunk
                    ci = colA[m]; colA[m] += 1
                    nc.scalar.activation(
                        pg[:, :alen], pg[:, :alen], AF.Exp,
                        bias=bias_n[:], scale=SCALE / (SE * SW),
                        accum_out=accA[m][:, ci:ci + 1])
                    blk += nb

            sred = sp.tile([128, 2 * m_tiles], FP32, tag="sred")
            for m in range(m_tiles):
                nc.vector.tensor_reduce(sred[:, m:m + 1],
                                        accA[m][:, :colA[m]], X, ALU.add)
                dcol = sred[:, m_tiles + m:m_tiles + m + 1]
                if colD[m] > 0:
                    nc.vector.tensor_reduce(dcol, accD[m][:, :colD[m]],
                                            X, ALU.add)
                else:
                    nc.vector.memset(dcol, 0.0)
            nc.sync.dma_start(out_d.ap()[:, :], sred[:])

    nc.compile()
    return nc


def make_in_maps(embeddings, weight, labels, b=B, cp=CP):
    """Host staging: normalize, fp8-cast, transpose, shard; plus the fp64
    label-correction context used by finalize()."""
    emb = np.asarray(embeddings, np.float64)
    w = np.asarray(weight, np.float64)
    lab = np.asarray(labels).astype(np.int64)
    c, d = w.shape

    ehat = emb / np.maximum(np.linalg.norm(emb, axis=1, keepdims=True), 1e-12)
    what = w / np.maximum(np.linalg.norm(w, axis=1, keepdims=True), 1e-12)

    e8 = (ehat * SE).astype(np.float32).astype(ml_dtypes.float8_e4m3)
    # [B, D] -> [B, K, 128] -> [128, K, B]
    ehT8 = np.ascontiguousarray(
        e8.reshape(b, K_CHUNKS, 128).transpose(2, 1, 0)).reshape(128, -1)

    w8 = (what * SW).astype(np.float32).astype(ml_dtypes.float8_e4m3)
    c_per = c // N_CORES
    in_maps = []
    for i in range(N_CORES):
        shard = np.zeros((cp, d), ml_dtypes.float8_e4m3)
        shard[:c_per] = w8[i * c_per:(i + 1) * c_per]
        # [CP, D] -> [NB, 512, K, 128] -> [NB, 128, K, 512] -> 2D
        wt8 = np.ascontiguousarray(
            shard.reshape(N_BLOCKS, CB, K_CHUNKS, 128).transpose(0, 3, 2, 1)
        ).reshape(N_BLOCKS * 128, K_CHUNKS * CB)
        in_maps.append({"ehT": ehT8, "wt": wt8})

    # fp64 label-column correction (exact cos at the label position)
    cos_l = np.einsum('bd,bd->b', ehat, what[lab])
    sin_l = np.sqrt(np.clip(1.0 - cos_l * cos_l, 0.0, 1.0))
    phi = cos_l * COS_M - sin_l * SIN_M
    phi = np.where(cos_l > TH, phi, cos_l - MM)
    t = SCALE * phi
    delta = np.exp(t + EXP_BIAS) - np.exp(SCALE * cos_l + EXP_BIAS)
    n_pad_total = (CP_EFF - c_per) * N_CORES
    host_ctx = {"t": t, "delta": delta,
                "pad": n_pad_total * math.exp(EXP_BIAS)}
    return in_maps, host_ctx


def finalize(core_outs, host_ctx, b=B):
    """core_outs: list of [128, 2*M_TILES] per-core partial sum-exp tiles
    (ACT-exact columns 0..7, DVE-Schraudolph columns 8..15)."""
    total = np.zeros((128, 2 * M_TILES), np.float64)
    for o in core_outs:
        total += np.asarray(o, np.float64)
    merged = total[:, :M_TILES] + total[:, M_TILES:]
    # row b = m*128 + p  ->  flatten [p, m] with order p-major per column
    sum_dev = merged.transpose(1, 0).reshape(b)
    sum_all = sum_dev + host_ctx["delta"] - host_ctx["pad"]
    loss = np.mean(np.log(sum_all) - EXP_BIAS - host_ctx["t"])
    return np.float32(loss)


_CACHED_NC = None


def kernel(embeddings, weight, labels):
    global _CACHED_NC
    if _CACHED_NC is None:
        _CACHED_NC = build_graph()
    in_maps, host_ctx = make_in_maps(embeddings, weight, labels)
    res = run_bass_kernel_spmd(_CACHED_NC, in_maps,
                               core_ids=list(range(N_CORES)), trace=False)
    return finalize([r["out"] for r in res.results], host_ctx)


if __name__ == "__main__":
    rng = np.random.default_rng(0)
    e = rng.standard_normal((B, D)).astype(np.float32)
    w = (rng.random((C, D)).astype(np.float32) - 0.5) * 0.015
    l = rng.integers(0, C, B).astype(np.int64)
    print(kernel(e, w, l))
